# revision 2
# baseline (speedup 1.0000x reference)
"""Trainium2 Bass kernel for BertWithAdaThresholdLocContextPooling (v4).

Pure data parallel over batch (B=16 -> 2 per core x 8 cores).

Host-side prep (data movement + dtype casts only): gathers mention rows of
sequence_output/attention with numpy fancy indexing, pre-transposes and
chunk-packs the weights, folds the extractor bias into a ner+ones chunk.

Numerics: ent path bf16; localized-context path fp8 (seq, ht*64, rs/4
against 4*W_rs) -- validated vs reference at rel err 4.8e-3.

DMA order (serialized chain): packed smalls | wner -> wx_ent p0 -> seq8 ->
wx_ent p1 -> wx_rs p0 -> wx_rs p1 -> wbs p0 -> wbs p1, with tensor-engine
program order matched so the PE is never waiting long on a late piece.
"""

import sys

for _p in ("/opt/trn_rl_repo",):
    if _p not in sys.path:
        sys.path.insert(0, _p)

import numpy as np
import ml_dtypes

import concourse.bacc as bacc
import concourse.bass as bass
import concourse.mybir as mybir
from concourse.tile import TileContext
from concourse.bass_utils import run_bass_kernel_spmd

F32 = mybir.dt.float32
BF16 = mybir.dt.bfloat16
F8 = mybir.dt.float8e4
AF = mybir.ActivationFunctionType
ALU = mybir.AluOpType
NPF8 = ml_dtypes.float8_e4m3
NPBF = ml_dtypes.bfloat16

B, L, HID = 16, 512, 768
HEADS, M = 12, 4
EMB, BLK, NER, NCLS = 768, 8, 6, 97
NCORES = 8
BPC = B // NCORES          # 2
CAT = 2 * HID + NER        # 1542
KCH = 12                   # 128-row contraction chunks (ent 0-5, rs 6-11)
NEMB = EMB // 128          # 6
NL = L // 128              # 4
NBL = EMB * BLK // 128     # 48

# CBF2 [128, 156] bf16 packed consts:
# selE [0:16,0:4] | w12 [0:12,4:5] | nert [0:7,5:9]
# | selA [0:96,9:53] (h cols +0:12, t cols +32:44)
# | bb row [0:1,53:150] | ones [0:1,150:152] | id4 [0:4,152:156]
SELE0, W120, NERT0, SELA0, BB0 = 0, 4, 5, 9, 53
ID40 = BB0 + NCLS + BPC
CBF2_COLS = ID40 + 4

_cache = {}


def _build_constants():
    cbf = np.zeros((128, CBF2_COLS), NPBF)
    for k in range(4 * M):
        cbf[k, SELE0 + k // M] = 1.0
    cbf[0:HEADS, W120] = 1.0 / HEADS
    for i in range(2):
        for m in range(M):
            for h in range(HEADS):
                cbf[i * M * HEADS + m * HEADS + h, SELA0 + i * 32 + h] = 1.0 / M

    rys = np.zeros((128, BLK * 128), NPBF)
    for y in range(BLK):
        for p in range(128):
            rys[(p // BLK) * BLK + y, y * 128 + p] = 1.0

    cbf[0:4, ID40:ID40 + 4] = np.eye(4).astype(NPBF)

    perm = np.empty(EMB * BLK, np.int64)
    for cch in range(NEMB):
        for y in range(BLK):
            for p in range(128):
                g = cch * 16 + p // BLK
                x = p % BLK
                perm[(cch * BLK + y) * 128 + p] = g * 64 + x * BLK + y
    return {"cbf": cbf, "rys": rys, "perm": perm}


def _build_program():
    nc = bacc.Bacc("TRN2", target_bir_lowering=False, debug=False)

    sg_h = nc.dram_tensor("sg", [4 * M, HID], BF16, kind="ExternalInput")
    at_h = nc.dram_tensor("at2", [2 * M * HEADS, BPC * L], BF16,
                          kind="ExternalInput")
    seq_h = nc.dram_tensor("seq", [128, BPC * NL * HID], F8, kind="ExternalInput")
    wxe_h = nc.dram_tensor("wxe", [128, 6 * 2 * EMB + BLK * 128], BF16,
                           kind="ExternalInput")
    wxr_h = nc.dram_tensor("wxr", [128, 6 * 2 * EMB], F8, kind="ExternalInput")
    wner_h = nc.dram_tensor("wner", [NER + 1, 2 * EMB], BF16, kind="ExternalInput")
    wbs_h = nc.dram_tensor("wbs", [128, NBL * NCLS], BF16, kind="ExternalInput")
    cbf_h = nc.dram_tensor("cbf", [128, CBF2_COLS], BF16, kind="ExternalInput")
    out_h = nc.dram_tensor("logitsT", [NCLS, BPC], F32, kind="ExternalOutput")

    with TileContext(nc) as tc:
        with (
            tc.tile_pool(name="const", bufs=1) as cp,
            tc.tile_pool(name="data", bufs=1) as dp,
            tc.tile_pool(name="psbig", bufs=1, space="PSUM") as psb,
            tc.tile_pool(name="psea", bufs=2, space="PSUM") as pse,
            tc.tile_pool(name="pssm", bufs=3, space="PSUM") as pss,
        ):
            from concourse.tile_rust import add_dep_helper

            # ---- all loads on the gpsimd queue in consumption order;
            # FIFO descriptor draining gives just-in-time arrival ----
            sg = dp.tile([4 * M, HID], BF16)
            nc.gpsimd.dma_start(sg[:], sg_h[:])
            at2 = dp.tile([2 * M * HEADS, BPC * L], BF16)
            nc.gpsimd.dma_start(at2[:], at_h[:])
            cbf = cp.tile([128, CBF2_COLS], BF16)
            nc.gpsimd.dma_start(cbf[:], cbf_h[:])

            selE = cbf[0:16, SELE0:SELE0 + 4]
            w12 = cbf[0:12, W120:W120 + 1]
            nert = cbf[0:NER + 1, NERT0:NERT0 + 2 * BPC]
            selA = cbf[0:96, SELA0:SELA0 + 44]
            bbrow = cbf[0:1, BB0:BB0 + NCLS]
            onesrow = cbf[0:1, BB0 + NCLS:BB0 + NCLS + BPC]
            id4 = cbf[0:4, ID40:ID40 + 4]

            wner = cp.tile([NER + 1, 2 * EMB], BF16)
            nc.gpsimd.dma_start(wner[:], wner_h[:])
            seqt = dp.tile([128, BPC * NL * HID], F8)
            nc.gpsimd.dma_start(seqt[:], seq_h[:])
            wxrt = cp.tile([128, 6 * 2 * EMB], F8)
            wxr = [wxrt[:, 0:3 * 2 * EMB], wxrt[:, 3 * 2 * EMB:]]
            wxe0 = cp.tile([128, 3 * 2 * EMB], BF16)
            wxe1 = cp.tile([128, 3 * 2 * EMB + BLK * 128], BF16)
            wxe = [wxe0, wxe1]
            rys = wxe1[:, 3 * 2 * EMB:]
            wbs = cp.tile([128, NBL * NCLS], BF16)
            nc.gpsimd.dma_start(wxrt[:], wxr_h[:])
            nc.gpsimd.dma_start(wxe0[:], wxe_h[:, 0:3 * 2 * EMB])
            nc.gpsimd.dma_start(wxe1[:], wxe_h[:, 3 * 2 * EMB:])
            nc.gpsimd.dma_start(wbs[:], wbs_h[:])

            # ---- activation table prewarm: Exp early (off critical path) ----
            junk = dp.tile([1, 1], F32)
            nc.scalar.activation(junk[:], cbf[0:1, 0:1], AF.Exp)
            # mention-exp early on the scalar queue (readers come later)
            exps = dp.tile([4 * M, HID], BF16)
            nc.scalar.activation(exps[:], sg[:], AF.Exp)

            # ---- extractor psums + first (ner+bias) chunk, then pools ----
            ps_rsc = pss.tile([128, NEMB * BPC], F32, tag="ry", bufs=1)
            pw = psb.tile([36, EMB], F32, tag="big")
            ph = pw[0:4, :]
            pt = pw[32:36, :]

            def extr_chunk(j, jj):
                if j == KCH:
                    lhsT = nert
                elif j < NEMB:
                    lhsT = entT[:, j * 4:(j + 1) * 4]
                else:
                    lhsT = rsc8[:, (j - NEMB) * 4:(j - NEMB + 1) * 4]
                for wi, tgt in ((0, ph), (1, pt)):
                    for n0, nl_ in ((0, 512), (512, 256)):
                        if j == KCH:
                            rhs = wner[:, wi * EMB + n0:wi * EMB + n0 + nl_]
                        elif j < NEMB:
                            pi, jo = j // 3, j % 3
                            base = jo * 2 * EMB + wi * EMB + n0
                            rhs = wxe[pi][:, base:base + nl_]
                        else:
                            base = (j - NEMB) * 2 * EMB + wi * EMB + n0
                            rhs = wxrt[:, base:base + nl_]
                        nc.tensor.matmul(tgt[:, n0:n0 + nl_], lhsT=lhsT, rhs=rhs,
                                         start=(jj == 0), stop=(jj == KCH))

            extr_chunk(KCH, 0)

            # ---- attention pooling + context vector prep (b1's heavy copies
            # interleave before b0's normalize chain) ----
            ps_ea, prd = [], []
            for b in range(BPC):
                pe_t = pse.tile([44, L], F32, tag="ea", name=f"ps_ea{b}")
                nc.tensor.matmul(pe_t[:], lhsT=selA,
                                 rhs=at2[:, b * L:(b + 1) * L],
                                 start=True, stop=True)
                eah = dp.tile([HEADS, L], F32, tag=f"eah{b}")
                nc.vector.tensor_copy(eah[:], pe_t[0:12, :])
                pr_t = dp.tile([HEADS, L], BF16, tag=f"prd{b}")
                nc.vector.tensor_tensor(out=pr_t[:], in0=eah[:],
                                        in1=pe_t[32:44, :], op=ALU.mult)
                ps_ea.append(pe_t)
                prd.append(pr_t)
            htc = []
            for b in range(BPC):
                ps_ht = pss.tile([1, L], F32, tag="sm", name=f"ps_ht{b}")
                nc.tensor.matmul(ps_ht[:], lhsT=w12, rhs=prd[b][:],
                                 start=True, stop=True)
                sm = dp.tile([1, 1], F32, tag=f"sm{b}")
                nc.vector.reduce_sum(sm[:], ps_ht[:], axis=mybir.AxisListType.X)
                r64 = dp.tile([1, 1], F32, tag=f"r64{b}")
                nc.vector.tensor_scalar(out=r64[:], in0=sm[:],
                                        scalar1=1.0 / 64.0, scalar2=1e-5 / 64.0,
                                        op0=ALU.mult, op1=ALU.add)
                rcp = dp.tile([1, 1], F32, tag=f"rcp{b}")
                nc.vector.reciprocal(rcp[:], r64[:])
                htn = dp.tile([1, L], BF16, tag=f"htn{b}")
                nc.vector.tensor_scalar_mul(htn[:], ps_ht[:], rcp[:, :1])
                ps_htc = pss.tile([128, NL * 2], BF16, tag="sm",
                                  name=f"ps_htc{b}")
                for c in range(NL):
                    nc.tensor.transpose(ps_htc[:, 2 * c:2 * c + 1],
                                        htn[:, c * 128:(c + 1) * 128],
                                        id4[0:1, 0:1])
                h = dp.tile([128, NL], F8, tag=f"htc{b}")
                nc.vector.tensor_copy(h[:], ps_htc[:, 0:NL * 2:2])
                htc.append(h)

            # ---- rs, entity, then the remaining extractor chunks ----
            def rs_batch(b):
                for d in range(NEMB):
                    for c in range(NL):
                        nc.tensor.matmul(
                            ps_rsc[:, d * BPC + b:d * BPC + b + 1],
                            lhsT=seqt[:, (b * NL + c) * HID + d * 128:
                                      (b * NL + c) * HID + (d + 1) * 128],
                            rhs=htc[b][:, c:c + 1],
                            start=(c == 0), stop=(c == NL - 1))

            # consumption order: ner+bias, rs chunks 6..11, ent chunks 0..5
            order = [KCH] + list(range(NEMB, KCH)) + list(range(NEMB))
            rs_batch(0)
            rs_batch(1)
            rsc8 = dp.tile([128, 4 * NEMB], F8)
            nc.vector.tensor_scalar_mul(
                rsc8[:].rearrange("p (r b m) -> p r b m", r=NEMB, b=BPC),
                ps_rsc[:].rearrange("p (r b) -> p r b", r=NEMB)
                .unsqueeze(3).broadcast_to([128, NEMB, BPC, 2]),
                1.0 / 256.0)

            # ---- entity embeddings: log-sum-exp over mentions (after the
            # rs section so its vector/scalar ops don't block rsc8) ----
            ps_e1 = pss.tile([4, 512], F32, tag="sm")
            ps_e2 = pss.tile([4, 256], F32, tag="sm")
            nc.tensor.matmul(ps_e1[:], lhsT=selE, rhs=exps[:, 0:512],
                             start=True, stop=True)
            nc.tensor.matmul(ps_e2[:], lhsT=selE, rhs=exps[:, 512:768],
                             start=True, stop=True)
            ent = dp.tile([4, EMB], BF16)
            nc.scalar.activation(ent[:, 0:512], ps_e1[:], AF.Ln)
            nc.scalar.activation(ent[:, 512:768], ps_e2[:], AF.Ln)
            # prewarm the tanh table while the extractor runs
            junk2 = dp.tile([1, 1], F32)
            nc.scalar.activation(junk2[:], cbf[0:1, 0:1], AF.Tanh)
            ps_et = pss.tile([128, 4 * NEMB], BF16, tag="sm")
            for c in range(NEMB):
                nc.tensor.transpose(ps_et[:, c * 4:(c + 1) * 4],
                                    ent[:, c * 128:(c + 1) * 128], id4)
            entT = dp.tile([128, 4 * NEMB], BF16)
            nc.vector.tensor_copy(entT[:], ps_et[:])

            for jj, j in enumerate(order[1:], 1):
                extr_chunk(j, jj)

            t4 = []
            for wi, ps_w in ((0, ph), (1, pt)):
                t = dp.tile([4, EMB], BF16, tag=f"t4_{wi}")
                nc.scalar.activation(t[:], ps_w[:], AF.Tanh)
                t4.append(t)

            # ---- transpose hs2/ts2 to columns ----
            ps_a = pss.tile([128, 4 * NEMB], BF16, tag="sm")
            ps_b2 = pss.tile([128, 4 * NEMB], BF16, tag="sm")
            for c in range(NEMB):
                nc.tensor.transpose(ps_a[:, c * 4:(c + 1) * 4],
                                    t4[0][:, c * 128:(c + 1) * 128], id4)
                nc.tensor.transpose(ps_b2[:, c * 4:(c + 1) * 4],
                                    t4[1][:, c * 128:(c + 1) * 128], id4)
            h2t = dp.tile([128, 4 * NEMB], BF16)
            nc.vector.tensor_copy(
                h2t[:].rearrange("p (c b) -> p c b", c=NEMB)[:, :, 0:4:2],
                ps_a[:].rearrange("p (c b) -> p c b", c=NEMB)[:, :, 0:4:2])
            nc.vector.tensor_copy(
                h2t[:].rearrange("p (c b) -> p c b", c=NEMB)[:, :, 1:4:2],
                ps_b2[:].rearrange("p (c b) -> p c b", c=NEMB)[:, :, 1:4:2])

            # ---- grouped bilinear + classifier ----
            ps_t2x = pss.tile([128, BLK * NEMB * BPC], F32, tag="sm")
            tscols = h2t[:].rearrange("p (c b) -> p c b", c=NEMB)[:, :, 1:4:2]
            for y in range(BLK):
                nc.tensor.matmul(
                    ps_t2x[:, y * 12:(y + 1) * 12]
                    .rearrange("p (c b) -> p c b", c=NEMB),
                    lhsT=rys[:, y * 128:(y + 1) * 128],
                    rhs=tscols, start=True, stop=True)
            blt = dp.tile([128, NEMB * 16], BF16)
            for c in range(NEMB):
                nc.vector.tensor_tensor(
                    out=blt[:, c * 16:(c + 1) * 16]
                    .rearrange("p (y b) -> p y b", y=BLK),
                    in0=h2t[:, c * 4:c * 4 + 4:2].unsqueeze(1)
                        .broadcast_to([128, BLK, 2]),
                    in1=ps_t2x[:].rearrange("p (y c b) -> p y c b", y=BLK, c=NEMB)
                    [:, :, c, :],
                    op=ALU.mult)
            ps_l = pss.tile([NCLS, BPC], F32, tag="sm")
            for c in range(NEMB):
                for y in range(BLK):
                    k = c * BLK + y
                    nc.tensor.matmul(ps_l[:], lhsT=wbs[:, k * NCLS:(k + 1) * NCLS],
                                     rhs=blt[:, c * 16 + y * 2:c * 16 + y * 2 + 2],
                                     start=(k == 0), stop=False)
            # bias as a rank-1 accumulation step: bb^T @ ones
            nc.tensor.matmul(ps_l[:], lhsT=bbrow, rhs=onesrow,
                             start=False, stop=True)
            lg = dp.tile([NCLS, BPC], F32)
            nc.vector.tensor_copy(lg[:], ps_l[:])
            nc.sync.dma_start(out_h[:], lg[:])

    nc.finalize()
    return nc


def _get_program():
    if "nc" not in _cache:
        _cache["nc"] = _build_program()
        _cache["consts"] = _build_constants()
    return _cache["nc"], _cache["consts"]


def kernel(sequence_output, attention, entity_pos, hs_ner_tags, ts_ner_tags,
           Wh, bh, Wt, bt, Wb, bb):
    nc, c = _get_program()

    seq = np.asarray(sequence_output, dtype=np.float32)
    attn = np.asarray(attention, dtype=np.float32)
    pos = np.asarray(entity_pos).astype(np.int64)
    starts = pos + 1                                        # [B, 2, M]
    nh = np.asarray(hs_ner_tags, dtype=np.float32)
    nt = np.asarray(ts_ner_tags, dtype=np.float32)

    # host-side gathers (data movement only)
    b_idx = np.arange(B)[:, None, None]
    sg_all = seq[b_idx, starts]                             # [B, 2, M, HID]
    at_all = attn.transpose(0, 2, 1, 3)[b_idx, starts]      # [B, 2, M, HEADS, L]
    at_all = at_all.reshape(B, 2 * M * HEADS, L)            # rows (i, m, h)

    # weights
    whT = np.ascontiguousarray(np.asarray(Wh, dtype=np.float32).T)   # [CAT, EMB]
    wtT = np.ascontiguousarray(np.asarray(Wt, dtype=np.float32).T)
    wxe = np.empty((128, 6 * 2 * EMB + BLK * 128), NPBF)
    wxe[:, 6 * 2 * EMB:] = c["rys"]
    wxei = wxe[:, 0:6 * 2 * EMB].reshape(128, 6, 2, EMB)
    wxr = np.empty((128, 6 * 2 * EMB), NPF8)
    wxri = wxr.reshape(128, 6, 2, EMB)
    for j in range(6):
        wxei[:, j, 0, :] = whT[j * 128:(j + 1) * 128]
        wxei[:, j, 1, :] = wtT[j * 128:(j + 1) * 128]
        r0 = 6 * 128 + j * 128
        wxri[:, j, 0, :] = (whT[r0:r0 + 128] * 4.0).astype(NPF8)
        wxri[:, j, 1, :] = (wtT[r0:r0 + 128] * 4.0).astype(NPF8)
    wner = np.empty((NER + 1, 2 * EMB), NPBF)
    wner[0:NER, 0:EMB] = whT[KCH * 128:CAT]
    wner[0:NER, EMB:] = wtT[KCH * 128:CAT]
    wner[NER, 0:EMB] = np.asarray(bh, np.float32)
    wner[NER, EMB:] = np.asarray(bt, np.float32)

    wbT = np.ascontiguousarray(np.asarray(Wb, dtype=np.float32).T)[c["perm"]]
    wbs = wbT.reshape(NBL, 128, NCLS).transpose(1, 0, 2).reshape(128, NBL * NCLS)
    wbs = np.ascontiguousarray(wbs).astype(NPBF)


    in_maps = []
    for core in range(NCORES):
        b0 = core * BPC
        sgc = sg_all[b0:b0 + BPC].reshape(4 * M, HID).astype(NPBF)
        seqim = np.empty((128, BPC * NL * HID), NPF8)
        for b in range(BPC):
            s = seq[b0 + b].reshape(NL, 128, HID).transpose(1, 0, 2)
            seqim[:, b * NL * HID:(b + 1) * NL * HID] = s.reshape(
                128, NL * HID).astype(NPF8)
        cbf = c["cbf"].copy()
        for b in range(BPC):
            cbf[0:NER, NERT0 + b * 2 + 0] = nh[b0 + b].astype(NPBF)
            cbf[0:NER, NERT0 + b * 2 + 1] = nt[b0 + b].astype(NPBF)
        cbf[NER, NERT0:NERT0 + 2 * BPC] = 1.0
        cbf[0, BB0:BB0 + NCLS] = np.asarray(bb, np.float32).astype(NPBF)
        cbf[0, BB0 + NCLS:BB0 + NCLS + BPC] = 1.0
        # at2 [96, BPC*L]: rows (i,m,h), cols b*L+l
        atc = np.ascontiguousarray(at_all[b0:b0 + BPC].transpose(
            1, 0, 2).reshape(2 * M * HEADS, BPC * L)).astype(NPBF)
        im = {
            "sg": np.ascontiguousarray(sgc),
            "at2": atc,
            "seq": seqim,
            "wxe": wxe, "wxr": wxr, "wner": wner, "wbs": wbs,
            "cbf": cbf,
        }
        in_maps.append(im)

    res = run_bass_kernel_spmd(nc, in_maps, core_ids=list(range(NCORES)))
    _cache["last_res"] = res
    out = np.empty((B, NCLS), np.float32)
    for core in range(NCORES):
        out[core * BPC:(core + 1) * BPC] = res.results[core]["logitsT"].T
    return out


# revision 3
# speedup vs baseline: 1.0087x; 1.0087x over previous
"""Trainium2 Bass kernel for BertWithAdaThresholdLocContextPooling (v4).

Pure data parallel over batch (B=16 -> 2 per core x 8 cores).

Host-side prep (data movement + dtype casts only): gathers mention rows of
sequence_output/attention with numpy fancy indexing, pre-transposes and
chunk-packs the weights, folds the extractor bias into a ner+ones chunk.

Numerics: ent path bf16; localized-context path fp8 (seq, ht*64, rs/4
against 4*W_rs) -- validated vs reference at rel err 4.8e-3.

DMA order (serialized chain): packed smalls | wner -> wx_ent p0 -> seq8 ->
wx_ent p1 -> wx_rs p0 -> wx_rs p1 -> wbs p0 -> wbs p1, with tensor-engine
program order matched so the PE is never waiting long on a late piece.
"""

import sys

for _p in ("/opt/trn_rl_repo",):
    if _p not in sys.path:
        sys.path.insert(0, _p)

import numpy as np
import ml_dtypes

import concourse.bacc as bacc
import concourse.bass as bass
import concourse.mybir as mybir
from concourse.tile import TileContext
from concourse.bass_utils import run_bass_kernel_spmd

F32 = mybir.dt.float32
BF16 = mybir.dt.bfloat16
F8 = mybir.dt.float8e4
AF = mybir.ActivationFunctionType
ALU = mybir.AluOpType
NPF8 = ml_dtypes.float8_e4m3
NPBF = ml_dtypes.bfloat16

B, L, HID = 16, 512, 768
HEADS, M = 12, 4
EMB, BLK, NER, NCLS = 768, 8, 6, 97
NCORES = 8
BPC = B // NCORES          # 2
CAT = 2 * HID + NER        # 1542
KCH = 12                   # 128-row contraction chunks (ent 0-5, rs 6-11)
NEMB = EMB // 128          # 6
NL = L // 128              # 4
NBL = EMB * BLK // 128     # 48

# CBF2 [128, 156] bf16 packed consts:
# selE [0:16,0:4] | w12 [0:12,4:5] | nert [0:7,5:9]
# | selA [0:96,9:53] (h cols +0:12, t cols +32:44)
# | bb row [0:1,53:150] | ones [0:1,150:152] | id4 [0:4,152:156]
SELE0, W120, NERT0, SELA0, BB0 = 0, 4, 5, 9, 53
ID40 = BB0 + NCLS + BPC
SELE96 = ID40 + 4
ID24 = SELE96 + 24
CBF2_COLS = ID24 + 24

_cache = {}


def _build_constants():
    cbf = np.zeros((128, CBF2_COLS), NPBF)
    for k in range(4 * M):
        cbf[k, SELE0 + k // M] = 1.0
    cbf[0:HEADS, W120] = 1.0 / HEADS
    for i in range(2):
        for m in range(M):
            for h in range(HEADS):
                cbf[i * M * HEADS + m * HEADS + h, SELA0 + i * 32 + h] = 1.0 / M

    rys = np.zeros((128, BLK * 128), NPBF)
    for y in range(BLK):
        for p in range(128):
            rys[(p // BLK) * BLK + y, y * 128 + p] = 1.0

    cbf[0:4, ID40:ID40 + 4] = np.eye(4).astype(NPBF)
    # selE96: row ((b*2+i)*4+m)*6+s -> col (b*2+i)*6+s (lse pool in packed layout)
    for bi in range(4):
        for m in range(M):
            for sx in range(6):
                cbf[(bi * M + m) * 6 + sx, SELE96 + bi * 6 + sx] = 1.0
    cbf[0:24, ID24:ID24 + 24] = np.eye(24).astype(NPBF)

    perm = np.empty(EMB * BLK, np.int64)
    for cch in range(NEMB):
        for y in range(BLK):
            for p in range(128):
                g = cch * 16 + p // BLK
                x = p % BLK
                perm[(cch * BLK + y) * 128 + p] = g * 64 + x * BLK + y
    return {"cbf": cbf, "rys": rys, "perm": perm}


def _build_program():
    nc = bacc.Bacc("TRN2", target_bir_lowering=False, debug=False)

    at_h = nc.dram_tensor("at2", [2 * M * HEADS, BPC * L + 128], BF16,
                          kind="ExternalInput")
    seq_h = nc.dram_tensor("seq", [128, BPC * NL * HID], F8, kind="ExternalInput")
    wxe_h = nc.dram_tensor("wxe", [128, 6 * 2 * EMB + BLK * 128], BF16,
                           kind="ExternalInput")
    wxr_h = nc.dram_tensor("wxr", [128, 6 * 2 * EMB], F8, kind="ExternalInput")
    wner_h = nc.dram_tensor("wner", [NER + 1, 2 * EMB], BF16, kind="ExternalInput")
    wbs_h = nc.dram_tensor("wbs", [128, NBL * NCLS], BF16, kind="ExternalInput")
    cbf_h = nc.dram_tensor("cbf", [128, CBF2_COLS], BF16, kind="ExternalInput")
    out_h = nc.dram_tensor("logitsT", [NCLS, BPC], F32, kind="ExternalOutput")

    with TileContext(nc) as tc:
        with (
            tc.tile_pool(name="const", bufs=1) as cp,
            tc.tile_pool(name="data", bufs=1) as dp,
            tc.tile_pool(name="psbig", bufs=1, space="PSUM") as psb,
            tc.tile_pool(name="psea", bufs=2, space="PSUM") as pse,
            tc.tile_pool(name="pssm", bufs=3, space="PSUM") as pss,
        ):
            from concourse.tile_rust import add_dep_helper

            # ---- all loads on the gpsimd queue in consumption order;
            # FIFO descriptor draining gives just-in-time arrival ----
            at2 = dp.tile([2 * M * HEADS, BPC * L + 128], BF16)
            nc.gpsimd.dma_start(at2[:], at_h[:])
            sg96 = at2[0:96, BPC * L:BPC * L + 128]
            cbf = cp.tile([128, CBF2_COLS], BF16)
            nc.gpsimd.dma_start(cbf[:], cbf_h[:])

            selE96 = cbf[0:96, SELE96:SELE96 + 24]
            id24 = cbf[0:24, ID24:ID24 + 24]
            w12 = cbf[0:12, W120:W120 + 1]
            nert = cbf[0:NER + 1, NERT0:NERT0 + 2 * BPC]
            selA = cbf[0:96, SELA0:SELA0 + 44]
            bbrow = cbf[0:1, BB0:BB0 + NCLS]
            onesrow = cbf[0:1, BB0 + NCLS:BB0 + NCLS + BPC]
            id4 = cbf[0:4, ID40:ID40 + 4]

            wner = cp.tile([NER + 1, 2 * EMB], BF16)
            nc.gpsimd.dma_start(wner[:], wner_h[:])
            seqt = dp.tile([128, BPC * NL * HID], F8)
            nc.gpsimd.dma_start(seqt[:], seq_h[:])
            wxrt = cp.tile([128, 6 * 2 * EMB], F8)
            wxr = [wxrt[:, 0:3 * 2 * EMB], wxrt[:, 3 * 2 * EMB:]]
            wxe0 = cp.tile([128, 3 * 2 * EMB], BF16)
            wxe1 = cp.tile([128, 3 * 2 * EMB + BLK * 128], BF16)
            wxe = [wxe0, wxe1]
            rys = wxe1[:, 3 * 2 * EMB:]
            wbs = cp.tile([128, NBL * NCLS], BF16)
            nc.gpsimd.dma_start(wxrt[:], wxr_h[:])
            nc.gpsimd.dma_start(wxe0[:], wxe_h[:, 0:3 * 2 * EMB])
            nc.gpsimd.dma_start(wxe1[:], wxe_h[:, 3 * 2 * EMB:])
            nc.gpsimd.dma_start(wbs[:], wbs_h[:])

            # ---- activation table prewarm: Exp early (off critical path) ----
            junk = dp.tile([1, 1], F32)
            nc.scalar.activation(junk[:], cbf[0:1, 0:1], AF.Exp)
            # mention-exp early on the scalar queue (readers come later)
            exps96 = dp.tile([96, 128], BF16)
            nc.scalar.activation(exps96[:], sg96, AF.Exp)

            # ---- extractor psums + first (ner+bias) chunk, then pools ----
            ps_rsc = pss.tile([128, NEMB * BPC], F32, tag="ry", bufs=1)
            pw = psb.tile([36, EMB], F32, tag="big")
            ph = pw[0:4, :]
            pt = pw[32:36, :]

            def extr_chunk(j, jj):
                if j == KCH:
                    lhsT = nert
                elif j < NEMB:
                    lhsT = entT[:, j:j + 19:6]
                else:
                    lhsT = rsc8[:, (j - NEMB) * 4:(j - NEMB + 1) * 4]
                for wi, tgt in ((0, ph), (1, pt)):
                    for n0, nl_ in ((0, 512), (512, 256)):
                        if j == KCH:
                            rhs = wner[:, wi * EMB + n0:wi * EMB + n0 + nl_]
                        elif j < NEMB:
                            pi, jo = j // 3, j % 3
                            base = jo * 2 * EMB + wi * EMB + n0
                            rhs = wxe[pi][:, base:base + nl_]
                        else:
                            base = (j - NEMB) * 2 * EMB + wi * EMB + n0
                            rhs = wxrt[:, base:base + nl_]
                        nc.tensor.matmul(tgt[:, n0:n0 + nl_], lhsT=lhsT, rhs=rhs,
                                         start=(jj == 0), stop=(jj == KCH))

            extr_chunk(KCH, 0)

            # ---- attention pooling + context vector prep (b1's heavy copies
            # interleave before b0's normalize chain) ----
            ps_ea, prd = [], []
            for b in range(BPC):
                pe_t = pse.tile([44, L], F32, tag="ea", name=f"ps_ea{b}")
                nc.tensor.matmul(pe_t[:], lhsT=selA,
                                 rhs=at2[:, b * L:(b + 1) * L],
                                 start=True, stop=True)
                eah = dp.tile([HEADS, L], F32, tag=f"eah{b}")
                nc.vector.tensor_copy(eah[:], pe_t[0:12, :])
                pr_t = dp.tile([HEADS, L], BF16, tag=f"prd{b}")
                nc.vector.tensor_tensor(out=pr_t[:], in0=eah[:],
                                        in1=pe_t[32:44, :], op=ALU.mult)
                ps_ea.append(pe_t)
                prd.append(pr_t)
            htc = []
            for b in range(BPC):
                ps_ht = pss.tile([1, L], F32, tag="sm", name=f"ps_ht{b}")
                nc.tensor.matmul(ps_ht[:], lhsT=w12, rhs=prd[b][:],
                                 start=True, stop=True)
                sm = dp.tile([1, 1], F32, tag=f"sm{b}")
                nc.vector.reduce_sum(sm[:], ps_ht[:], axis=mybir.AxisListType.X)
                r64 = dp.tile([1, 1], F32, tag=f"r64{b}")
                nc.vector.tensor_scalar(out=r64[:], in0=sm[:],
                                        scalar1=1.0 / 64.0, scalar2=1e-5 / 64.0,
                                        op0=ALU.mult, op1=ALU.add)
                rcp = dp.tile([1, 1], F32, tag=f"rcp{b}")
                nc.vector.reciprocal(rcp[:], r64[:])
                htn = dp.tile([1, L], BF16, tag=f"htn{b}")
                nc.vector.tensor_scalar_mul(htn[:], ps_ht[:], rcp[:, :1])
                ps_htc = pss.tile([128, NL * 2], BF16, tag="sm",
                                  name=f"ps_htc{b}")
                for c in range(NL):
                    nc.tensor.transpose(ps_htc[:, 2 * c:2 * c + 1],
                                        htn[:, c * 128:(c + 1) * 128],
                                        id4[0:1, 0:1])
                h = dp.tile([128, NL], F8, tag=f"htc{b}")
                nc.vector.tensor_copy(h[:], ps_htc[:, 0:NL * 2:2])
                htc.append(h)

            # ---- rs, entity, then the remaining extractor chunks ----
            def rs_batch(b):
                for d in range(NEMB):
                    for c in range(NL):
                        nc.tensor.matmul(
                            ps_rsc[:, d * BPC + b:d * BPC + b + 1],
                            lhsT=seqt[:, (b * NL + c) * HID + d * 128:
                                      (b * NL + c) * HID + (d + 1) * 128],
                            rhs=htc[b][:, c:c + 1],
                            start=(c == 0), stop=(c == NL - 1))

            # consumption order: ner+bias, rs chunks 6..11, ent chunks 0..5
            order = [KCH] + list(range(NEMB, KCH)) + list(range(NEMB))
            rs_batch(0)
            rs_batch(1)
            rsc8 = dp.tile([128, 4 * NEMB], F8)
            nc.vector.tensor_scalar_mul(
                rsc8[:].rearrange("p (r b m) -> p r b m", r=NEMB, b=BPC),
                ps_rsc[:].rearrange("p (r b) -> p r b", r=NEMB)
                .unsqueeze(3).broadcast_to([128, NEMB, BPC, 2]),
                1.0 / 256.0)

            # ---- entity embeddings: log-sum-exp over mentions (after the
            # rs section so its vector/scalar ops don't block rsc8) ----
            ps_e96 = pss.tile([24, 128], F32, tag="sm")
            nc.tensor.matmul(ps_e96[:], lhsT=selE96, rhs=exps96[:],
                             start=True, stop=True)
            ent96 = dp.tile([24, 128], BF16)
            nc.scalar.activation(ent96[:], ps_e96[:], AF.Ln)
            # prewarm the tanh table while the extractor runs
            junk2 = dp.tile([1, 1], F32)
            nc.scalar.activation(junk2[:], cbf[0:1, 0:1], AF.Tanh)
            ps_etT = pss.tile([128, 24], BF16, tag="sm")
            nc.tensor.transpose(ps_etT[:], ent96[:], id24)
            entT = dp.tile([128, 24], BF16)
            nc.vector.tensor_copy(entT[:], ps_etT[:])

            for jj, j in enumerate(order[1:], 1):
                extr_chunk(j, jj)

            t4 = []
            for wi, ps_w in ((0, ph), (1, pt)):
                t = dp.tile([4, EMB], BF16, tag=f"t4_{wi}")
                nc.scalar.activation(t[:], ps_w[:], AF.Tanh)
                t4.append(t)

            # ---- transpose hs2/ts2 to columns ----
            ps_a = pss.tile([128, 4 * NEMB], BF16, tag="sm")
            ps_b2 = pss.tile([128, 4 * NEMB], BF16, tag="sm")
            for c in range(NEMB):
                nc.tensor.transpose(ps_a[:, c * 4:(c + 1) * 4],
                                    t4[0][:, c * 128:(c + 1) * 128], id4)
                nc.tensor.transpose(ps_b2[:, c * 4:(c + 1) * 4],
                                    t4[1][:, c * 128:(c + 1) * 128], id4)
            h2t = dp.tile([128, 4 * NEMB], BF16)
            nc.vector.tensor_copy(
                h2t[:].rearrange("p (c b) -> p c b", c=NEMB)[:, :, 0:4:2],
                ps_a[:].rearrange("p (c b) -> p c b", c=NEMB)[:, :, 0:4:2])
            nc.vector.tensor_copy(
                h2t[:].rearrange("p (c b) -> p c b", c=NEMB)[:, :, 1:4:2],
                ps_b2[:].rearrange("p (c b) -> p c b", c=NEMB)[:, :, 1:4:2])

            # ---- grouped bilinear + classifier ----
            ps_t2x = pss.tile([128, BLK * NEMB * BPC], F32, tag="sm")
            tscols = h2t[:].rearrange("p (c b) -> p c b", c=NEMB)[:, :, 1:4:2]
            for y in range(BLK):
                nc.tensor.matmul(
                    ps_t2x[:, y * 12:(y + 1) * 12]
                    .rearrange("p (c b) -> p c b", c=NEMB),
                    lhsT=rys[:, y * 128:(y + 1) * 128],
                    rhs=tscols, start=True, stop=True)
            blt = dp.tile([128, NEMB * 16], BF16)
            for c in range(NEMB):
                nc.vector.tensor_tensor(
                    out=blt[:, c * 16:(c + 1) * 16]
                    .rearrange("p (y b) -> p y b", y=BLK),
                    in0=h2t[:, c * 4:c * 4 + 4:2].unsqueeze(1)
                        .broadcast_to([128, BLK, 2]),
                    in1=ps_t2x[:].rearrange("p (y c b) -> p y c b", y=BLK, c=NEMB)
                    [:, :, c, :],
                    op=ALU.mult)
            ps_l = pss.tile([NCLS, BPC], F32, tag="sm")
            for c in range(NEMB):
                for y in range(BLK):
                    k = c * BLK + y
                    nc.tensor.matmul(ps_l[:], lhsT=wbs[:, k * NCLS:(k + 1) * NCLS],
                                     rhs=blt[:, c * 16 + y * 2:c * 16 + y * 2 + 2],
                                     start=(k == 0), stop=False)
            # bias as a rank-1 accumulation step: bb^T @ ones
            nc.tensor.matmul(ps_l[:], lhsT=bbrow, rhs=onesrow,
                             start=False, stop=True)
            lg = dp.tile([NCLS, BPC], F32)
            nc.vector.tensor_copy(lg[:], ps_l[:])
            nc.sync.dma_start(out_h[:], lg[:])

    nc.finalize()
    return nc


def _get_program():
    if "nc" not in _cache:
        _cache["nc"] = _build_program()
        _cache["consts"] = _build_constants()
    return _cache["nc"], _cache["consts"]


def kernel(sequence_output, attention, entity_pos, hs_ner_tags, ts_ner_tags,
           Wh, bh, Wt, bt, Wb, bb):
    nc, c = _get_program()

    seq = np.asarray(sequence_output, dtype=np.float32)
    attn = np.asarray(attention, dtype=np.float32)
    pos = np.asarray(entity_pos).astype(np.int64)
    starts = pos + 1                                        # [B, 2, M]
    nh = np.asarray(hs_ner_tags, dtype=np.float32)
    nt = np.asarray(ts_ner_tags, dtype=np.float32)

    # host-side gathers (data movement only)
    b_idx = np.arange(B)[:, None, None]
    sg_all = seq[b_idx, starts]                             # [B, 2, M, HID]
    at_all = attn.transpose(0, 2, 1, 3)[b_idx, starts]      # [B, 2, M, HEADS, L]
    at_all = at_all.reshape(B, 2 * M * HEADS, L)            # rows (i, m, h)

    # weights
    whT = np.ascontiguousarray(np.asarray(Wh, dtype=np.float32).T)   # [CAT, EMB]
    wtT = np.ascontiguousarray(np.asarray(Wt, dtype=np.float32).T)
    wxe = np.empty((128, 6 * 2 * EMB + BLK * 128), NPBF)
    wxe[:, 6 * 2 * EMB:] = c["rys"]
    wxei = wxe[:, 0:6 * 2 * EMB].reshape(128, 6, 2, EMB)
    wxr = np.empty((128, 6 * 2 * EMB), NPF8)
    wxri = wxr.reshape(128, 6, 2, EMB)
    for j in range(6):
        wxei[:, j, 0, :] = whT[j * 128:(j + 1) * 128]
        wxei[:, j, 1, :] = wtT[j * 128:(j + 1) * 128]
        r0 = 6 * 128 + j * 128
        wxri[:, j, 0, :] = (whT[r0:r0 + 128] * 4.0).astype(NPF8)
        wxri[:, j, 1, :] = (wtT[r0:r0 + 128] * 4.0).astype(NPF8)
    wner = np.empty((NER + 1, 2 * EMB), NPBF)
    wner[0:NER, 0:EMB] = whT[KCH * 128:CAT]
    wner[0:NER, EMB:] = wtT[KCH * 128:CAT]
    wner[NER, 0:EMB] = np.asarray(bh, np.float32)
    wner[NER, EMB:] = np.asarray(bt, np.float32)

    wbT = np.ascontiguousarray(np.asarray(Wb, dtype=np.float32).T)[c["perm"]]
    wbs = wbT.reshape(NBL, 128, NCLS).transpose(1, 0, 2).reshape(128, NBL * NCLS)
    wbs = np.ascontiguousarray(wbs).astype(NPBF)


    in_maps = []
    for core in range(NCORES):
        b0 = core * BPC
        seqim = np.empty((128, BPC * NL * HID), NPF8)
        for b in range(BPC):
            s = seq[b0 + b].reshape(NL, 128, HID).transpose(1, 0, 2)
            seqim[:, b * NL * HID:(b + 1) * NL * HID] = s.reshape(
                128, NL * HID).astype(NPF8)
        cbf = c["cbf"].copy()
        for b in range(BPC):
            cbf[0:NER, NERT0 + b * 2 + 0] = nh[b0 + b].astype(NPBF)
            cbf[0:NER, NERT0 + b * 2 + 1] = nt[b0 + b].astype(NPBF)
        cbf[NER, NERT0:NERT0 + 2 * BPC] = 1.0
        cbf[0, BB0:BB0 + NCLS] = np.asarray(bb, np.float32).astype(NPBF)
        cbf[0, BB0 + NCLS:BB0 + NCLS + BPC] = 1.0
        # at2 [96, BPC*L+128]: attention rows (i,m,h) | sg packed [96,128]
        atc = np.empty((2 * M * HEADS, BPC * L + 128), NPBF)
        atc[:, 0:BPC * L] = at_all[b0:b0 + BPC].transpose(1, 0, 2).reshape(
            2 * M * HEADS, BPC * L).astype(NPBF)
        atc[:, BPC * L:] = sg_all[b0:b0 + BPC].reshape(96, 128).astype(NPBF)
        im = {
            "at2": atc,
            "seq": seqim,
            "wxe": wxe, "wxr": wxr, "wner": wner, "wbs": wbs,
            "cbf": cbf,
        }
        in_maps.append(im)

    res = run_bass_kernel_spmd(nc, in_maps, core_ids=list(range(NCORES)))
    _cache["last_res"] = res
    out = np.empty((B, NCLS), np.float32)
    for core in range(NCORES):
        out[core * BPC:(core + 1) * BPC] = res.results[core]["logitsT"].T
    return out


# revision 4
# speedup vs baseline: 1.0300x; 1.0211x over previous
"""Trainium2 Bass kernel for BertWithAdaThresholdLocContextPooling (v4).

Pure data parallel over batch (B=16 -> 2 per core x 8 cores).

Host-side prep (data movement + dtype casts only): gathers mention rows of
sequence_output/attention with numpy fancy indexing, pre-transposes and
chunk-packs the weights, folds the extractor bias into a ner+ones chunk.

Numerics: ent path bf16; localized-context path fp8 (seq, ht*64, rs/4
against 4*W_rs) -- validated vs reference at rel err 4.8e-3.

DMA order (serialized chain): packed smalls | wner -> wx_ent p0 -> seq8 ->
wx_ent p1 -> wx_rs p0 -> wx_rs p1 -> wbs p0 -> wbs p1, with tensor-engine
program order matched so the PE is never waiting long on a late piece.
"""

import sys

for _p in ("/opt/trn_rl_repo",):
    if _p not in sys.path:
        sys.path.insert(0, _p)

import numpy as np
import ml_dtypes

import concourse.bacc as bacc
import concourse.bass as bass
import concourse.mybir as mybir
from concourse.tile import TileContext
from concourse.bass_utils import run_bass_kernel_spmd

F32 = mybir.dt.float32
BF16 = mybir.dt.bfloat16
F8 = mybir.dt.float8e4
AF = mybir.ActivationFunctionType
ALU = mybir.AluOpType
NPF8 = ml_dtypes.float8_e4m3
NPBF = ml_dtypes.bfloat16

B, L, HID = 16, 512, 768
HEADS, M = 12, 4
EMB, BLK, NER, NCLS = 768, 8, 6, 97
NCORES = 8
BPC = B // NCORES          # 2
CAT = 2 * HID + NER        # 1542
KCH = 12                   # 128-row contraction chunks (ent 0-5, rs 6-11)
NEMB = EMB // 128          # 6
NL = L // 128              # 4
NBL = EMB * BLK // 128     # 48

# CBF2 [128, 156] bf16 packed consts:
# selE [0:16,0:4] | w12 [0:12,4:5] | nert [0:7,5:9]
# | selA [0:96,9:53] (h cols +0:12, t cols +32:44)
# | bb row [0:1,53:150] | ones [0:1,150:152] | id4 [0:4,152:156]
SELE0, W120, NERT0, SELA0, BB0 = 0, 4, 5, 9, 53
ID40 = BB0 + NCLS + BPC
SELE96 = ID40 + 4
ID24 = SELE96 + 24
CBF2_COLS = ID24 + 24

_cache = {}


def _build_constants():
    cbf = np.zeros((128, CBF2_COLS), NPBF)
    for k in range(4 * M):
        cbf[k, SELE0 + k // M] = 1.0
    cbf[0:HEADS, W120] = 1.0 / HEADS
    for i in range(2):
        for m in range(M):
            for h in range(HEADS):
                cbf[i * M * HEADS + m * HEADS + h, SELA0 + i * 32 + h] = 1.0 / M

    rys = np.zeros((128, BLK * 128), NPBF)
    for y in range(BLK):
        for p in range(128):
            rys[(p // BLK) * BLK + y, y * 128 + p] = 1.0

    cbf[0:4, ID40:ID40 + 4] = np.eye(4).astype(NPBF)
    # selE96: row ((b*2+i)*4+m)*6+s -> col (b*2+i)*6+s (lse pool in packed layout)
    for bi in range(4):
        for m in range(M):
            for sx in range(6):
                cbf[(bi * M + m) * 6 + sx, SELE96 + bi * 6 + sx] = 1.0
    cbf[0:24, ID24:ID24 + 24] = np.eye(24).astype(NPBF)

    perm = np.empty(EMB * BLK, np.int64)
    for cch in range(NEMB):
        for y in range(BLK):
            for p in range(128):
                g = cch * 16 + p // BLK
                x = p % BLK
                perm[(cch * BLK + y) * 128 + p] = g * 64 + x * BLK + y
    return {"cbf": cbf, "rys": rys, "perm": perm}


def _build_program():
    nc = bacc.Bacc("TRN2", target_bir_lowering=False, debug=False)

    at_h = nc.dram_tensor("at2", [2 * M * HEADS, BPC * L + 128], BF16,
                          kind="ExternalInput")
    seq_h = nc.dram_tensor("seq", [128, BPC * NL * HID], F8, kind="ExternalInput")
    wxe_h = nc.dram_tensor("wxe", [128, 6 * 2 * EMB + BLK * 128], BF16,
                           kind="ExternalInput")
    wxr_h = nc.dram_tensor("wxr", [128, 6 * 2 * EMB], F8, kind="ExternalInput")
    wner_h = nc.dram_tensor("wner", [NER + 1, 2 * EMB], BF16, kind="ExternalInput")
    wbs_h = nc.dram_tensor("wbs", [128, NBL * NCLS], BF16, kind="ExternalInput")
    cbf_h = nc.dram_tensor("cbf", [128, CBF2_COLS], BF16, kind="ExternalInput")
    out_h = nc.dram_tensor("logitsT", [NCLS, BPC], F32, kind="ExternalOutput")

    with TileContext(nc) as tc:
        with (
            tc.tile_pool(name="const", bufs=1) as cp,
            tc.tile_pool(name="data", bufs=1) as dp,
            tc.tile_pool(name="psbig", bufs=1, space="PSUM") as psb,
            tc.tile_pool(name="psea", bufs=2, space="PSUM") as pse,
            tc.tile_pool(name="pssm", bufs=3, space="PSUM") as pss,
        ):
            from concourse.tile_rust import add_dep_helper

            # ---- all loads on the gpsimd queue in consumption order;
            # FIFO descriptor draining gives just-in-time arrival ----
            at2 = dp.tile([2 * M * HEADS, BPC * L + 128], BF16)
            nc.gpsimd.dma_start(at2[:], at_h[:])
            sg96 = at2[0:96, BPC * L:BPC * L + 128]
            cbf = cp.tile([128, CBF2_COLS], BF16)
            nc.gpsimd.dma_start(cbf[:], cbf_h[:])

            selE96 = cbf[0:96, SELE96:SELE96 + 24]
            id24 = cbf[0:24, ID24:ID24 + 24]
            w12 = cbf[0:12, W120:W120 + 1]
            nert = cbf[0:NER + 1, NERT0:NERT0 + 2 * BPC]
            selA = cbf[0:96, SELA0:SELA0 + 44]
            bbrow = cbf[0:1, BB0:BB0 + NCLS]
            onesrow = cbf[0:1, BB0 + NCLS:BB0 + NCLS + BPC]
            id4 = cbf[0:4, ID40:ID40 + 4]

            wner = cp.tile([NER + 1, 2 * EMB], BF16)
            nc.gpsimd.dma_start(wner[:], wner_h[:])
            seqt = dp.tile([128, BPC * NL * HID], F8)
            nc.gpsimd.dma_start(seqt[:], seq_h[:])
            wxrt = cp.tile([128, 6 * 2 * EMB], F8)
            wxr = [wxrt[:, 0:3 * 2 * EMB], wxrt[:, 3 * 2 * EMB:]]
            wxe0 = cp.tile([128, 3 * 2 * EMB], BF16)
            wxe1 = cp.tile([128, 3 * 2 * EMB + BLK * 128], BF16)
            wxe = [wxe0, wxe1]
            rys = wxe1[:, 3 * 2 * EMB:]
            wbs = cp.tile([128, NBL * NCLS], BF16)
            nc.gpsimd.dma_start(wxrt[:], wxr_h[:])
            nc.gpsimd.dma_start(wxe0[:], wxe_h[:, 0:3 * 2 * EMB])
            nc.gpsimd.dma_start(wxe1[:], wxe_h[:, 3 * 2 * EMB:])
            nc.gpsimd.dma_start(wbs[:], wbs_h[:])

            # ---- activation table prewarm: Exp early (off critical path) ----
            junk = dp.tile([1, 1], F32)
            nc.scalar.activation(junk[:], cbf[0:1, 0:1], AF.Exp)
            # mention-exp early on the scalar queue (readers come later)
            exps96 = dp.tile([96, 128], BF16)
            nc.scalar.activation(exps96[:], sg96, AF.Exp)

            # ---- extractor psums + first (ner+bias) chunk, then pools ----
            ps_rsc = pss.tile([128, NEMB * BPC], F32, tag="ry", bufs=1)
            pw = psb.tile([36, EMB], F32, tag="big")
            ph = pw[0:4, :]
            pt = pw[32:36, :]

            def extr_chunk(j, jj):
                if j == KCH:
                    lhsT = nert
                elif j < NEMB:
                    lhsT = entT[:, j:j + 19:6]
                else:
                    lhsT = rsc8[:, (j - NEMB) * 4:(j - NEMB + 1) * 4]
                for wi, tgt in ((0, ph), (1, pt)):
                    for n0, nl_ in ((0, 512), (512, 256)):
                        if j == KCH:
                            rhs = wner[:, wi * EMB + n0:wi * EMB + n0 + nl_]
                        elif j < NEMB:
                            pi, jo = j // 3, j % 3
                            base = jo * 2 * EMB + wi * EMB + n0
                            rhs = wxe[pi][:, base:base + nl_]
                        else:
                            base = (j - NEMB) * 2 * EMB + wi * EMB + n0
                            rhs = wxrt[:, base:base + nl_]
                        nc.tensor.matmul(tgt[:, n0:n0 + nl_], lhsT=lhsT, rhs=rhs,
                                         start=(jj == 0), stop=(jj == KCH))

            extr_chunk(KCH, 0)

            def rs_batch(b):
                for d in range(NEMB):
                    for c in range(NL):
                        nc.tensor.matmul(
                            ps_rsc[:, d * BPC + b:d * BPC + b + 1],
                            lhsT=seqt[:, (b * NL + c) * HID + d * 128:
                                      (b * NL + c) * HID + (d + 1) * 128],
                            rhs=htc[b][:, c:c + 1],
                            start=(c == 0), stop=(c == NL - 1))

            # ---- attention pooling + context vector prep (b1's heavy copies
            # interleave before b0's normalize chain) ----
            ps_ea, prd = [], []
            for b in range(BPC):
                pe_t = pse.tile([44, L], F32, tag="ea", name=f"ps_ea{b}")
                nc.tensor.matmul(pe_t[:], lhsT=selA,
                                 rhs=at2[:, b * L:(b + 1) * L],
                                 start=True, stop=True)
                eah = dp.tile([HEADS, L], F32, tag=f"eah{b}")
                nc.vector.tensor_copy(eah[:], pe_t[0:12, :])
                pr_t = dp.tile([HEADS, L], BF16, tag=f"prd{b}")
                nc.vector.tensor_tensor(out=pr_t[:], in0=eah[:],
                                        in1=pe_t[32:44, :], op=ALU.mult)
                ps_ea.append(pe_t)
                prd.append(pr_t)
            htc = []
            for b in range(BPC):
                ps_ht = pss.tile([1, L], F32, tag="sm", name=f"ps_ht{b}")
                nc.tensor.matmul(ps_ht[:], lhsT=w12, rhs=prd[b][:],
                                 start=True, stop=True)
                sm = dp.tile([1, 1], F32, tag=f"sm{b}")
                nc.vector.reduce_sum(sm[:], ps_ht[:], axis=mybir.AxisListType.X)
                r64 = dp.tile([1, 1], F32, tag=f"r64{b}")
                nc.vector.tensor_scalar(out=r64[:], in0=sm[:],
                                        scalar1=1.0 / 64.0, scalar2=1e-5 / 64.0,
                                        op0=ALU.mult, op1=ALU.add)
                rcp = dp.tile([1, 1], F32, tag=f"rcp{b}")
                nc.vector.reciprocal(rcp[:], r64[:])
                htn = dp.tile([1, L], BF16, tag=f"htn{b}")
                nc.vector.tensor_scalar_mul(htn[:], ps_ht[:], rcp[:, :1])
                ps_htc = pss.tile([128, NL * 2], BF16, tag="sm",
                                  name=f"ps_htc{b}")
                for c in range(NL):
                    nc.tensor.transpose(ps_htc[:, 2 * c:2 * c + 1],
                                        htn[:, c * 128:(c + 1) * 128],
                                        id4[0:1, 0:1])
                h = dp.tile([128, NL], F8, tag=f"htc{b}")
                nc.vector.tensor_copy(h[:], ps_htc[:, 0:NL * 2:2])
                htc.append(h)
                rs_batch(b)

            # consumption order: ner+bias, rs chunks 6..11, ent chunks 0..5
            order = [KCH] + list(range(NEMB, KCH)) + list(range(NEMB))
            rsc8 = dp.tile([128, 4 * NEMB], F8)
            nc.vector.tensor_scalar_mul(
                rsc8[:].rearrange("p (r b m) -> p r b m", r=NEMB, b=BPC),
                ps_rsc[:].rearrange("p (r b) -> p r b", r=NEMB)
                .unsqueeze(3).broadcast_to([128, NEMB, BPC, 2]),
                1.0 / 256.0)

            # ---- entity embeddings: log-sum-exp over mentions (after the
            # rs section so its vector/scalar ops don't block rsc8) ----
            ps_e96 = pss.tile([24, 128], F32, tag="sm")
            nc.tensor.matmul(ps_e96[:], lhsT=selE96, rhs=exps96[:],
                             start=True, stop=True)
            ent96 = dp.tile([24, 128], BF16)
            nc.scalar.activation(ent96[:], ps_e96[:], AF.Ln)
            # prewarm the tanh table while the extractor runs
            junk2 = dp.tile([1, 1], F32)
            nc.scalar.activation(junk2[:], cbf[0:1, 0:1], AF.Tanh)
            ps_etT = pss.tile([128, 24], BF16, tag="sm")
            nc.tensor.transpose(ps_etT[:], ent96[:], id24)
            entT = dp.tile([128, 24], BF16)
            nc.vector.tensor_copy(entT[:], ps_etT[:])

            for jj, j in enumerate(order[1:], 1):
                extr_chunk(j, jj)

            t4 = []
            for wi, ps_w in ((0, ph), (1, pt)):
                t = dp.tile([4, EMB], BF16, tag=f"t4_{wi}")
                nc.scalar.activation(t[:], ps_w[:], AF.Tanh)
                t4.append(t)

            # ---- transpose hs2/ts2 to columns ----
            ps_a = pss.tile([128, 4 * NEMB], BF16, tag="sm")
            ps_b2 = pss.tile([128, 4 * NEMB], BF16, tag="sm")
            for c in range(NEMB):
                nc.tensor.transpose(ps_a[:, c * 4:(c + 1) * 4],
                                    t4[0][:, c * 128:(c + 1) * 128], id4)
                nc.tensor.transpose(ps_b2[:, c * 4:(c + 1) * 4],
                                    t4[1][:, c * 128:(c + 1) * 128], id4)
            h2t = dp.tile([128, 4 * NEMB], BF16)
            nc.vector.tensor_copy(
                h2t[:].rearrange("p (c b) -> p c b", c=NEMB)[:, :, 0:4:2],
                ps_a[:].rearrange("p (c b) -> p c b", c=NEMB)[:, :, 0:4:2])
            nc.vector.tensor_copy(
                h2t[:].rearrange("p (c b) -> p c b", c=NEMB)[:, :, 1:4:2],
                ps_b2[:].rearrange("p (c b) -> p c b", c=NEMB)[:, :, 1:4:2])

            # ---- grouped bilinear + classifier ----
            ps_t2x = pss.tile([128, BLK * NEMB * BPC], F32, tag="sm")
            tscols = h2t[:].rearrange("p (c b) -> p c b", c=NEMB)[:, :, 1:4:2]
            for y in range(BLK):
                nc.tensor.matmul(
                    ps_t2x[:, y * 12:(y + 1) * 12]
                    .rearrange("p (c b) -> p c b", c=NEMB),
                    lhsT=rys[:, y * 128:(y + 1) * 128],
                    rhs=tscols, start=True, stop=True)
            blt = dp.tile([128, NEMB * 16], BF16)
            for c in range(NEMB):
                nc.vector.tensor_tensor(
                    out=blt[:, c * 16:(c + 1) * 16]
                    .rearrange("p (y b) -> p y b", y=BLK),
                    in0=h2t[:, c * 4:c * 4 + 4:2].unsqueeze(1)
                        .broadcast_to([128, BLK, 2]),
                    in1=ps_t2x[:].rearrange("p (y c b) -> p y c b", y=BLK, c=NEMB)
                    [:, :, c, :],
                    op=ALU.mult)
            ps_l = pss.tile([NCLS, BPC], F32, tag="sm")
            for c in range(NEMB):
                for y in range(BLK):
                    k = c * BLK + y
                    nc.tensor.matmul(ps_l[:], lhsT=wbs[:, k * NCLS:(k + 1) * NCLS],
                                     rhs=blt[:, c * 16 + y * 2:c * 16 + y * 2 + 2],
                                     start=(k == 0), stop=False)
            # bias as a rank-1 accumulation step: bb^T @ ones
            nc.tensor.matmul(ps_l[:], lhsT=bbrow, rhs=onesrow,
                             start=False, stop=True)
            lg = dp.tile([NCLS, BPC], F32)
            nc.vector.tensor_copy(lg[:], ps_l[:])
            nc.sync.dma_start(out_h[:], lg[:])

    nc.finalize()
    return nc


def _get_program():
    if "nc" not in _cache:
        _cache["nc"] = _build_program()
        _cache["consts"] = _build_constants()
    return _cache["nc"], _cache["consts"]


def kernel(sequence_output, attention, entity_pos, hs_ner_tags, ts_ner_tags,
           Wh, bh, Wt, bt, Wb, bb):
    nc, c = _get_program()

    seq = np.asarray(sequence_output, dtype=np.float32)
    attn = np.asarray(attention, dtype=np.float32)
    pos = np.asarray(entity_pos).astype(np.int64)
    starts = pos + 1                                        # [B, 2, M]
    nh = np.asarray(hs_ner_tags, dtype=np.float32)
    nt = np.asarray(ts_ner_tags, dtype=np.float32)

    # host-side gathers (data movement only)
    b_idx = np.arange(B)[:, None, None]
    sg_all = seq[b_idx, starts]                             # [B, 2, M, HID]
    at_all = attn.transpose(0, 2, 1, 3)[b_idx, starts]      # [B, 2, M, HEADS, L]
    at_all = at_all.reshape(B, 2 * M * HEADS, L)            # rows (i, m, h)

    # weights
    whT = np.ascontiguousarray(np.asarray(Wh, dtype=np.float32).T)   # [CAT, EMB]
    wtT = np.ascontiguousarray(np.asarray(Wt, dtype=np.float32).T)
    wxe = np.empty((128, 6 * 2 * EMB + BLK * 128), NPBF)
    wxe[:, 6 * 2 * EMB:] = c["rys"]
    wxei = wxe[:, 0:6 * 2 * EMB].reshape(128, 6, 2, EMB)
    wxr = np.empty((128, 6 * 2 * EMB), NPF8)
    wxri = wxr.reshape(128, 6, 2, EMB)
    for j in range(6):
        wxei[:, j, 0, :] = whT[j * 128:(j + 1) * 128]
        wxei[:, j, 1, :] = wtT[j * 128:(j + 1) * 128]
        r0 = 6 * 128 + j * 128
        wxri[:, j, 0, :] = (whT[r0:r0 + 128] * 4.0).astype(NPF8)
        wxri[:, j, 1, :] = (wtT[r0:r0 + 128] * 4.0).astype(NPF8)
    wner = np.empty((NER + 1, 2 * EMB), NPBF)
    wner[0:NER, 0:EMB] = whT[KCH * 128:CAT]
    wner[0:NER, EMB:] = wtT[KCH * 128:CAT]
    wner[NER, 0:EMB] = np.asarray(bh, np.float32)
    wner[NER, EMB:] = np.asarray(bt, np.float32)

    wbT = np.ascontiguousarray(np.asarray(Wb, dtype=np.float32).T)[c["perm"]]
    wbs = wbT.reshape(NBL, 128, NCLS).transpose(1, 0, 2).reshape(128, NBL * NCLS)
    wbs = np.ascontiguousarray(wbs).astype(NPBF)


    in_maps = []
    for core in range(NCORES):
        b0 = core * BPC
        seqim = np.empty((128, BPC * NL * HID), NPF8)
        for b in range(BPC):
            s = seq[b0 + b].reshape(NL, 128, HID).transpose(1, 0, 2)
            seqim[:, b * NL * HID:(b + 1) * NL * HID] = s.reshape(
                128, NL * HID).astype(NPF8)
        cbf = c["cbf"].copy()
        for b in range(BPC):
            cbf[0:NER, NERT0 + b * 2 + 0] = nh[b0 + b].astype(NPBF)
            cbf[0:NER, NERT0 + b * 2 + 1] = nt[b0 + b].astype(NPBF)
        cbf[NER, NERT0:NERT0 + 2 * BPC] = 1.0
        cbf[0, BB0:BB0 + NCLS] = np.asarray(bb, np.float32).astype(NPBF)
        cbf[0, BB0 + NCLS:BB0 + NCLS + BPC] = 1.0
        # at2 [96, BPC*L+128]: attention rows (i,m,h) | sg packed [96,128]
        atc = np.empty((2 * M * HEADS, BPC * L + 128), NPBF)
        atc[:, 0:BPC * L] = at_all[b0:b0 + BPC].transpose(1, 0, 2).reshape(
            2 * M * HEADS, BPC * L).astype(NPBF)
        atc[:, BPC * L:] = sg_all[b0:b0 + BPC].reshape(96, 128).astype(NPBF)
        im = {
            "at2": atc,
            "seq": seqim,
            "wxe": wxe, "wxr": wxr, "wner": wner, "wbs": wbs,
            "cbf": cbf,
        }
        in_maps.append(im)

    res = run_bass_kernel_spmd(nc, in_maps, core_ids=list(range(NCORES)))
    _cache["last_res"] = res
    out = np.empty((B, NCLS), np.float32)
    for core in range(NCORES):
        out[core * BPC:(core + 1) * BPC] = res.results[core]["logitsT"].T
    return out
